# revision 48
# baseline (speedup 1.0000x reference)
"""Chamfer loss kernel for Trainium2, batch-parallel over 8 NeuronCores.

Per core (one batch element b):
  gts = src_points[b] @ R^T + t          (on device, f32r matmul)
  P[i,j] = |gts_i|^2 + |recon_j|^2 - 2 gts_i . recon_j
  loss_b = sum_j min_i P + sum_i min_j P
Host sums the 8 partial losses.

Structure:
- All matmuls run in float32r (fp32 operands at bf16-rate streaming), so
  the distance matmul needs no hi/lo decomposition at all.  The host
  folds the -2 into the transform, so the device pipeline is just:
  transform -> squares -> norm row -> one augmented distance matmul.
- The augmented operands put xx / yy / ones on 32-aligned partition rows
  (k=0..3 coords+ones, k=32 xx|ones, k=64 ones|yy, everything between
  zeroed) because ACT copies must write at 32-aligned partition bases and
  extra K rows are free on the PE (cost is column-count bound).
- ACT stages each PSUM tile to bf16 SBUF (pure dtype copy, no bias).
- The staged bf16 tiles are reduced by DVE in 2x mode: two running
  elementwise-min accumulators for the per-column mins (even/odd blocks,
  so DVE isn't serialized on one dependency chain), and a batched binary
  min-tree (two row blocks at a time) for the per-row mins.
- Per-column mins are finished with PE transposes + free-axis folds, and
  everything is summed with a final ones-matmul across partitions.
"""

import os

# the axon client here has no NTFF profile hook; a stray BASS_TRACE=1 in the
# environment would crash run_bass_kernel_spmd on a missing import
os.environ["BASS_NEVER_TRACE"] = "1"

import ml_dtypes
import numpy as np

import concourse.bacc as bacc
import concourse.bass as bass
import concourse.mybir as mybir
import concourse.tile as tile
from concourse.bass_utils import run_bass_kernel_spmd

F32 = mybir.dt.float32
F32R = mybir.dt.float32r
BF16 = mybir.dt.bfloat16
ALU = mybir.AluOpType
AX = mybir.AxisListType
AF = mybir.ActivationFunctionType

N_CORES = 8
NPTS = 4096          # points per set (both gts and recon)
NBLK = NPTS // 128   # 32 row blocks
HALF = 2048          # P tile free width (4 PSUM banks)
KA = 34              # augmented operand rows (0-3, 32-33 used)

_CACHE = {}
LAST_RESULTS = None


def _build_kernel():
    nc = bacc.Bacc("TRN2", target_bir_lowering=False, debug=False)

    srcT = nc.declare_dram_parameter("srcT", [4, NPTS], F32, isOutput=False)
    reconT = nc.declare_dram_parameter("reconT", [4, NPTS], F32, isOutput=False)
    taug = nc.declare_dram_parameter("taug", [4, 4], F32, isOutput=False)
    ident = nc.declare_dram_parameter("ident", [128, 128], BF16, isOutput=False)
    cnorm = nc.declare_dram_parameter("cnorm", [8, 4], F32, isOutput=False)
    cones = nc.declare_dram_parameter("cones", [128, 1], F32, isOutput=False)
    zeros = nc.declare_dram_parameter("zeros", [29, NPTS], F32, isOutput=False)
    loss = nc.declare_dram_parameter("loss", [1, 1], F32, isOutput=True)

    with tile.TileContext(nc) as tc:
        with tc.tile_pool(name="sb", bufs=1) as sb:
            prep_pool = tc.alloc_tile_pool(name="prep", bufs=1)
            # ---- phase 0: loads + operand-shell init --------------------
            taug_sb = sb.tile([4, 4], F32R)
            nc.sync.dma_start(out=taug_sb[:, :], in_=taug[:, :].bitcast(F32R))
            pts = prep_pool.tile([8, NPTS], F32R) # 0-3 src_aug, 4-7 recon_aug
            # interleave chunks across the SP and ACT DMA queues (each
            # queue's transfers serialize)
            for c in range(4):
                cs = slice(c * 1024, (c + 1) * 1024)
                eng = nc.sync if c % 2 == 0 else nc.scalar
                eng.dma_start(out=pts[0:4, cs], in_=srcT[:, cs].bitcast(F32R))
            for c in range(4):
                cs = slice(c * 1024, (c + 1) * 1024)
                eng = nc.scalar if c % 2 == 0 else nc.sync
                eng.dma_start(out=pts[4:8, cs], in_=reconT[:, cs].bitcast(F32R))
            # recon side of the distance operand, queued before the misc
            # loads (DMA is exempt from partition-base alignment)
            rhs_early = True
            ident_sb = sb.tile([128, 128], BF16)
            nc.sync.dma_start(out=ident_sb[:, :], in_=ident[:, :])
            norm_ones = sb.tile([8, 4], F32R)
            nc.sync.dma_start(out=norm_ones[:, :], in_=cnorm[:, :].bitcast(F32R))
            ones128 = sb.tile([128, 1], F32)
            nc.sync.dma_start(out=ones128[:, :], in_=cones[:, :])

            # augmented distance-matmul operands; zero the unused K rows so
            # they contribute nothing (both sides zeroed: no 0*garbage NaNs)
            lhs = sb.tile([KA, NPTS], F32R)   # 0-2 -2g, 3 ones*, 32 xx, 33 one
            rhs = sb.tile([KA, NPTS], F32R)   # 0-2 p, 3-31 zero, 32 one, 33 yy
            nc.sync.dma_start(out=rhs[0:3, :], in_=pts[4:7, :])
            zsrc = zeros[:, :].bitcast(F32R)
            nc.sync.dma_start(out=lhs[4:32, :], in_=zsrc[0:28, :])
            nc.sync.dma_start(out=rhs[3:32, :], in_=zsrc[0:29, :])

            # PE warm-up: tiny matmuls on the identity while inputs load,
            # so the transform/norm matmuls run at full PE clock
            with tc.tile_pool(name="warm_ps", bufs=1, space="PSUM") as wpp:
                warm_ps = wpp.tile([128, 128], F32)
                for _ in range(18):
                    nc.tensor.matmul(warm_ps[:, :], lhsT=ident_sb[:, :],
                                     rhs=ident_sb[:, :], start=True,
                                     stop=True)



            # ---- phase 1: transform + norms -----------------------------
            # squares of all 8 rows early (gts rows are dummies for now);
            # the first norm matmul's outputs only weight the recon rows,
            # so [ones, yy] is valid before the transform lands.
            # The transform and norm PSUM pools are HALF-width (8KB per
            # partition each) so they coexist in PSUM and the transform
            # can start as soon as the src points land, without waiting
            # for the norm pool to release its banks.
            sq = prep_pool.tile([8, NPTS], F32R)
            sq2 = prep_pool.tile([4, NPTS], F32R)
            nc.scalar.activation(sq[:, :], pts[:, :], AF.Square)
            with tc.tile_pool(name="gts_ps", bufs=1, space="PSUM") as gpp, \
                 tc.tile_pool(name="nrm_ps", bufs=1, space="PSUM") as npp:
                QW = 1024
                # transform: rows 0-2 = -2*gts (host folded -2 into taug),
                # row 3 = ones; copied straight into the lhs operand.
                # Quarter-width double-buffered tiles so quarter k+1's
                # matmuls overlap quarter k's copies.
                for hh in range(4):
                    hs = slice(hh * QW, (hh + 1) * QW)
                    g_t = gpp.tile([4, QW], F32, tag="G", bufs=2)
                    for c in range(QW // 512):
                        cs = slice(hh * QW + c * 512, hh * QW + (c + 1) * 512)
                        nc.tensor.matmul(g_t[:, c * 512:(c + 1) * 512],
                                         lhsT=taug_sb[:, :],
                                         rhs=pts[0:4, cs], start=True,
                                         stop=True)
                    nc.scalar.copy(lhs[0:4, hs], g_t[:, :])
                    # square on the idle DVE from the SBUF copy:
                    # (lhs*1.0)*lhs, keeping ACT free for the next copies
                    nc.vector.scalar_tensor_tensor(sq2[0:4, hs], lhs[0:4, hs],
                                                   1.0, lhs[0:4, hs],
                                                   ALU.mult, ALU.mult)
                # xx = 0.25 * sum((-2g)^2) + exact ones row, reusing the
                # transform pool's PSUM space per quarter
                for hh in range(4):
                    hs = slice(hh * QW, (hh + 1) * QW)
                    x_t = gpp.tile([4, QW], F32, tag="G", bufs=2)
                    for c in range(QW // 512):
                        cs = slice(hh * QW + c * 512, hh * QW + (c + 1) * 512)
                        nc.tensor.matmul(x_t[0:2, c * 512:(c + 1) * 512],
                                         lhsT=norm_ones[0:4, 2:4],
                                         rhs=sq2[0:4, cs], start=True,
                                         stop=True)
                    nc.scalar.copy(lhs[32:34, hs], x_t[0:2, :])  # xx; ones
                # [ones; yy] from the recon squares
                for hh in range(4):
                    hs = slice(hh * QW, (hh + 1) * QW)
                    n_t = npp.tile([2, QW], F32, tag="N", bufs=2)
                    for c in range(QW // 512):
                        cs = slice(hh * QW + c * 512, hh * QW + (c + 1) * 512)
                        nc.tensor.matmul(n_t[:, c * 512:(c + 1) * 512],
                                         lhsT=norm_ones[0:8, 0:2],
                                         rhs=sq[:, cs], start=True,
                                         stop=True)
                    # DVE is idle during prep; base 32 is a legal base
                    nc.vector.tensor_copy(rhs[32:34, hs], n_t[:, :])     # yy

            prep_pool.release()

            # ---- phase 3: distance tiles + min reductions ---------------
            rmin = sb.tile([128, NBLK], F32)        # per-block row mins
            mrun0 = sb.tile([128, NPTS], BF16)      # col-min over even blocks
            mrun1 = sb.tile([128, NPTS], BF16)      # col-min over odd blocks

            with tc.tile_pool(name="stage_sb", bufs=3) as stg, \
                 tc.tile_pool(name="main_ps", bufs=2, space="PSUM") as mps:
                batches = [(0, 2), (2, 2)] + [(4 * k, 4)
                                              for k in range(1, NBLK // 4)]
                for b0, nb in batches:
                    # stage nb row blocks, then one batched tree (first two
                    # batches are 2-wide so tree work starts early enough
                    # to fill DVE pipeline-fill gaps)
                    pbfull = stg.tile([128, 4 * NPTS], BF16, tag="PSB",
                                      bufs=2)
                    pb = pbfull[:, 0:nb * NPTS]
                    for q in range(nb):
                        ib = b0 + q
                        lw = lhs[0:KA, ib * 128:(ib + 1) * 128]
                        for h in range(2):
                            pt = mps.tile([128, HALF], F32, tag="P")
                            for s in range(HALF // 512):
                                j0 = h * HALF + s * 512
                                nc.tensor.matmul(
                                    pt[:, s * 512:(s + 1) * 512], lhsT=lw,
                                    rhs=rhs[0:KA, j0:j0 + 512],
                                    start=True, stop=True)
                            # stage to bf16 SBUF (pure dtype-convert copy)
                            nc.scalar.copy(
                                pb[:, q * NPTS + h * HALF:
                                   q * NPTS + (h + 1) * HALF], pt[:, :])
                        # running col-min (dual accumulators so the two
                        # merge chains schedule independently on DVE)
                        pslice = pb[:, q * NPTS:(q + 1) * NPTS]
                        mr = mrun0 if q % 2 == 0 else mrun1
                        if ib < 2:
                            nc.vector.tensor_copy(mr[:, :], pslice)
                        else:
                            nc.vector.tensor_tensor(mr[:, :], pslice,
                                                    mr[:, :], ALU.min)
                    # batched row-min tree: [128, nb, w] views
                    w = HALF
                    tr = pb.rearrange("p (b h w) -> p b h w", b=nb, h=2)
                    lvl = 0
                    while w >= 128:
                        ntf = stg.tile([128, 4, w], BF16,
                                       tag=f"TR{lvl}", bufs=2,
                                       name=f"tr{lvl}")
                        nt = ntf[:, 0:nb, :]
                        nc.vector.tensor_tensor(nt[:, :, :], tr[:, :, 0, :],
                                                tr[:, :, 1, :], ALU.min)
                        tr = nt.rearrange("p b (h w) -> p b h w", h=2)
                        w //= 2
                        lvl += 1
                    # one more 2x TT level before the (1x) reduce
                    ntf2 = stg.tile([128, 4, 64], BF16, tag="TRF", bufs=2,
                                    name="trf")
                    nt2 = ntf2[:, 0:nb, :]
                    nc.vector.tensor_tensor(nt2[:, :, :], tr[:, :, 0, :],
                                            tr[:, :, 1, :], ALU.min)
                    nc.vector.tensor_reduce(
                        rmin[:, b0:b0 + nb], nt2, axis=AX.X, op=ALU.min)

            # ---- phase 4: finishers -------------------------------------
            mrun = sb.tile([128, NPTS], BF16)
            rsum = sb.tile([128, 1], F32)
            cmin = sb.tile([128, NPTS // 128], F32)
            csum = sb.tile([128, 1], F32)
            tot = sb.tile([128, 1], F32)
            loss_sb = sb.tile([1, 1], F32)

            nc.vector.tensor_tensor(mrun[:, :], mrun0[:, :], mrun1[:, :],
                                    ALU.min)
            nc.vector.tensor_reduce(rsum[:, :], rmin[:, :], axis=AX.X,
                                    op=ALU.add)

            with tc.tile_pool(name="fin_ps", bufs=4, space="PSUM") as fps:
                # 8 transposes per PSUM tile, one batched fold per group
                for g in range(NPTS // 1024):
                    tp = fps.tile([128, 1024], BF16, tag="T")
                    for c in range(8):
                        j0 = (g * 8 + c) * 128
                        nc.tensor.transpose(tp[:, c * 128:(c + 1) * 128],
                                            mrun[:, j0:j0 + 128],
                                            ident_sb[:, :])
                    nc.vector.tensor_reduce(
                        cmin[:, 8 * g:8 * g + 8],
                        tp.rearrange("p (g w) -> p g w", w=128),
                        axis=AX.X, op=ALU.min)
                nc.vector.tensor_reduce(csum[:, :], cmin[:, :], axis=AX.X,
                                        op=ALU.add)
                nc.vector.tensor_tensor(tot[:, :], rsum[:, :], csum[:, :],
                                        ALU.add)

                loss_ps = fps.tile([1, 1], F32, tag="L", bufs=1)
                nc.tensor.matmul(loss_ps[:, :], lhsT=tot[:, :],
                                 rhs=ones128[:, :], start=True, stop=True)
                nc.scalar.copy(loss_sb[:, :], loss_ps[:, :])

            nc.sync.dma_start(out=loss[:, :], in_=loss_sb[:, :])

    nc.compile()
    return nc


def _prep_core_inputs(recon_b, src_b, transform_b):
    src_aug = np.empty((4, NPTS), np.float32)
    src_aug[0:3] = src_b.T
    src_aug[3] = 1.0
    rec_aug = np.empty((4, NPTS), np.float32)
    rec_aug[0:3] = recon_b.T
    rec_aug[3] = 1.0
    R = transform_b[:3, :3]
    t = transform_b[:3, 3]
    # -2 folded into the transform: device rows are -2*gts, and the xx
    # ones-matmul weights are 0.25 to undo the square of the -2
    ta = np.zeros((4, 4), np.float32)
    ta[0:3, 0:3] = -2.0 * R.T
    ta[3, 0:3] = -2.0 * t
    ta[3, 3] = 1.0
    # nrm_ps rows = [cnorm col0 . sq, col1 . sq] -> rhs[32:34] = [ones, yy]
    # nrm2_ps rows = [col2 . sq_gts, col3 . sq_gts] -> lhs[32:34] = [xx, ones]
    cnorm = np.zeros((8, 4), np.float32)
    cnorm[7, 0] = 1.0      # col 0 -> ones (recon aug row squared)
    cnorm[4:7, 1] = 1.0    # col 1 -> yy
    cnorm[0:3, 2] = 0.25   # col 2 -> xx from (-2*gts)^2
    cnorm[3, 3] = 1.0      # col 3 -> ones (gts aug row squared)
    return {
        "srcT": np.ascontiguousarray(src_aug),
        "reconT": np.ascontiguousarray(rec_aug),
        "taug": ta,
        "ident": np.eye(128).astype(ml_dtypes.bfloat16),
        "cnorm": cnorm,
        "cones": np.ones((128, 1), np.float32),
        "zeros": np.zeros((29, NPTS), np.float32),
    }


def kernel(recon, src_points, transform):
    global LAST_RESULTS
    recon = np.asarray(recon, np.float32)
    src_points = np.asarray(src_points, np.float32)
    transform = np.asarray(transform, np.float32)
    B = recon.shape[0]
    assert B == N_CORES

    if "nc" not in _CACHE:
        _CACHE["nc"] = _build_kernel()
    nc = _CACHE["nc"]

    in_maps = [
        _prep_core_inputs(recon[b], src_points[b], transform[b])
        for b in range(B)
    ]
    res = run_bass_kernel_spmd(nc, in_maps, list(range(N_CORES)))
    LAST_RESULTS = res
    total = np.float64(0.0)
    for r in res.results:
        total += np.float64(r["loss"][0, 0])
    return np.float32(total)
